# revision 20
# baseline (speedup 1.0000x reference)
"""Causal multi-head self-attention with RoPE on 8 Trainium2 NeuronCores.

Sharding: batch (4) x head-half (2) -> 8 self-contained cores. Each core
computes Q/K/V projections for its 8 heads, RoPE, causal flash-style
attention (scores kept transposed [key, query] so probs feed the V matmul
with no on-device transpose), and a partial output projection over its 512
context features. The two partial outputs per batch are summed on host
(the "all-reduce after output projection" of the tensor-parallel split).

v2 layout notes:
- The attention kj-stream for super-block sb is ACT(exp)-bound; projection
  matmuls for sb+1 and the deferred output projection of sb-1 are emitted
  interleaved into it (paced units) so PE stays dense while ACT drains exp.
- Q/K (and the RoPE tables' sin half) are bf16: same PE rate, half the DVE
  rope cost on the two pure-bf16 ops, half the SBUF.
- Softmax normalization per head-pair is emitted right after its last PV
  matmul: one wide reciprocal (both pars), two PE broadcasts, two DVE muls
  reading PSUM directly (no ACT relay copy).
- Scores are masked additively via a PE-written -1e30 triangle and
  exponentiated without max-subtraction (scores are bounded); per-query sums
  come free from a ones-column appended to V.
"""

import sys

sys.path.insert(0, "/opt/trn_rl_repo")

import numpy as np

B, S_FULL, D, H = 4, 2048, 1024, 16
DK = 64  # head dim
HL = 8  # heads per core
DL = HL * DK  # 512 local features
ROPE_THETA = 10000.0
NEG = -1.0e30

_CACHE = {}


def _emit(nc, tc, tensors, S, reps=1):
    import concourse.tile as tile  # noqa: F401
    from concourse import mybir
    from contextlib import ExitStack
    from collections import deque

    f32, f32r = mybir.dt.float32, mybir.dt.float32r
    bf16 = mybir.dt.bfloat16
    AF = mybir.ActivationFunctionType
    SWAP = [i ^ 1 for i in range(32)]
    NSB = S // 512  # query super-blocks
    NKB = S // 128  # key blocks
    DEPTH = 3  # kj-iterations the V-matmul trails the score matmul by

    xT, wqT, wkT, wvT, woT = (
        tensors["xT"], tensors["wqT"], tensors["wkT"], tensors["wvT"], tensors["woT"],
    )
    tabsc, tabss = tensors["tabsc"], tensors["tabss"]
    maskT, ident = tensors["maskT"], tensors["ident"]
    ones65, outp = tensors["ones65"], tensors["outp"]

    with ExitStack() as ctx:
        const = ctx.enter_context(tc.tile_pool(name="const", bufs=1))
        wres = ctx.enter_context(tc.tile_pool(name="wres", bufs=1))
        xt_p = ctx.enter_context(tc.tile_pool(name="xt", bufs=9))
        tb_p = ctx.enter_context(tc.tile_pool(name="tb", bufs=2))
        kt_p = ctx.enter_context(tc.tile_pool(name="kt", bufs=1))
        vt_p = ctx.enter_context(tc.tile_pool(name="vt", bufs=1))
        qt_p = ctx.enter_context(tc.tile_pool(name="qt", bufs=2))
        qs_p = ctx.enter_context(tc.tile_pool(name="qs", bufs=2))
        ex_p = ctx.enter_context(tc.tile_pool(name="ex", bufs=DEPTH + 1))
        cx_p = ctx.enter_context(tc.tile_pool(name="cx", bufs=2))
        rc_p = ctx.enter_context(tc.tile_pool(name="rc", bufs=2))
        rl_p = ctx.enter_context(tc.tile_pool(name="rl", bufs=1))
        os_p = ctx.enter_context(tc.tile_pool(name="os", bufs=3))
        pp = ctx.enter_context(tc.tile_pool(name="pp", bufs=2, space="PSUM"))
        ps = ctx.enter_context(tc.tile_pool(name="ps", bufs=4, space="PSUM"))
        pc = ctx.enter_context(tc.tile_pool(name="pc", bufs=1, space="PSUM"))

        woT_r = woT.rearrange("(t p) o -> p t o", p=128)

        # -- initial loads: wk on sync first (first PE work), x on gpsimd --
        def emit_proj_dmas(sb, all_gpsimd=False):
            s0 = sb * 512
            xts = []
            for ct in range(8):
                xt_t = xt_p.tile([128, 512], f32r, tag="xt", name="xt")
                eng = nc.gpsimd if (all_gpsimd or ct % 2 == 0) else nc.sync
                eng.dma_start(
                    xt_t[:], xT[ct * 128 : (ct + 1) * 128, s0 : s0 + 512].bitcast(f32r)
                )
                xts.append(xt_t)
            tbc_t = tb_p.tile([128, 512], f32, tag="tbc", name="tbc")
            nc.sync.dma_start(tbc_t[:], tabsc[:, s0 : s0 + 512])
            tbs_t = tb_p.tile([128, 512], bf16, tag="tbs", name="tbs")
            nc.sync.dma_start(tbs_t[:], tabss[:, s0 : s0 + 512])
            return xts, tbc_t, tbs_t

        wq_t = wres.tile([128, 8, DL], f32r, tag="wq")
        wk_t = wres.tile([128, 8, DL], f32r, tag="wk")
        wv_t = wres.tile([128, 8, DL], f32r, tag="wv")
        wo_t = wres.tile([128, 4, D], f32r, tag="wo")
        for ct in range(8):
            nc.sync.dma_start(
                wk_t[:, ct, :], wkT[ct * 128 : (ct + 1) * 128, :].bitcast(f32r)
            )
        pd0 = emit_proj_dmas(0, all_gpsimd=True)
        for ct in range(8):
            nc.sync.dma_start(
                wq_t[:, ct, :], wqT[ct * 128 : (ct + 1) * 128, :].bitcast(f32r)
            )
        for ct in range(8):
            nc.gpsimd.dma_start(
                wv_t[:, ct, :], wvT[ct * 128 : (ct + 1) * 128, :].bitcast(f32r)
            )
        maskT_t = const.tile([128, 896], bf16, tag="maskT")
        nc.sync.dma_start(maskT_t[:], maskT[:])
        ident_t = const.tile([128, 128], bf16, tag="ident")
        nc.sync.dma_start(ident_t[:], ident[:])
        ones_t = const.tile([65, 64], f32r, tag="ones")
        nc.sync.dma_start(ones_t[:], ones65[:].bitcast(f32r))
        for it in range(4):
            nc.sync.dma_start(wo_t[:, it, :], woT_r[:, it, :].bitcast(f32r))

        # persistent K / V buffers
        kt_tiles = {}
        for hp in range(4):
            for sbk in range(NSB):
                kt_tiles[hp, sbk] = kt_p.tile(
                    [128, 512], bf16, tag=f"kt{hp}_{sbk}", name=f"kt{hp}_{sbk}"
                )
        v_tiles = {}
        for kb in range(NKB):
            v_tiles[kb] = vt_p.tile([128, HL, 65], f32r, tag=f"v{kb}", name=f"v{kb}")
            nc.vector.memset(v_tiles[kb][:, :, 64:65].bitcast(f32), 1.0)

        # -- deferred-work units, injected into the attention stream --
        units = deque()

        def inject(n):
            for _ in range(n):
                if units:
                    units.popleft()()

        qt_cur = {}
        pr_store = {}

        def push_proj_units(sb, xts, tbc_t, tbs_t):
            def rope(pr, dst):
                qs_t = qs_p.tile([128, 512], f32, tag="qs", name="qs")
                nc.vector.stream_shuffle(qs_t[:], pr[:], SWAP)
                nc.vector.tensor_mul(dst[:], pr[:], tbc_t[:])
                qb_t = qs_p.tile([128, 512], bf16, tag="qb", name="qb")
                nc.vector.tensor_mul(qb_t[:], qs_t[:], tbs_t[:])
                nc.vector.tensor_add(dst[:], dst[:], qb_t[:])

            for w_t, is_k in ((wk_t, True), (wq_t, False)):
                for ft in range(4):
                    key = (sb, is_k, ft)

                    def ua(key=key, w_t=w_t, ft=ft, xts=xts):
                        pr = pp.tile([128, 512], f32, tag="mm", name="pr")
                        pr_store[key] = pr
                        for j, ct in enumerate((0, 2, 4, 6)):
                            nc.tensor.matmul(
                                pr[:],
                                w_t[:, ct, ft * 128 : (ft + 1) * 128],
                                xts[ct][:],
                                start=(j == 0),
                                stop=False,
                            )

                    def ub(key=key, w_t=w_t, ft=ft, xts=xts, is_k=is_k, sb=sb):
                        pr = pr_store.pop(key)
                        for j, ct in enumerate((1, 3, 5, 7)):
                            nc.tensor.matmul(
                                pr[:],
                                w_t[:, ct, ft * 128 : (ft + 1) * 128],
                                xts[ct][:],
                                start=False,
                                stop=(j == 3),
                            )
                        if is_k:
                            dst = kt_tiles[ft, sb]
                        else:
                            dst = qt_p.tile(
                                [128, 512], bf16, tag=f"qt{ft}", name=f"qt{ft}"
                            )
                            qt_cur[ft] = dst
                        rope(pr, dst)

                    units.append(ua)
                    units.append(ub)
            for i in range(4):
                kb = sb * 4 + i
                key = (sb, "v", i)

                def va(key=key, i=i, xts=xts):
                    pr = pp.tile([128, 512], f32, tag="mm", name="vp")
                    pr_store[key] = pr
                    for ct in range(4):
                        nc.tensor.matmul(
                            pr[:],
                            xts[ct][:, i * 128 : (i + 1) * 128],
                            wv_t[:, ct, :],
                            start=(ct == 0),
                            stop=False,
                        )

                def vb(key=key, i=i, xts=xts, kb=kb):
                    pr = pr_store.pop(key)
                    for ct in range(4, 8):
                        nc.tensor.matmul(
                            pr[:],
                            xts[ct][:, i * 128 : (i + 1) * 128],
                            wv_t[:, ct, :],
                            start=False,
                            stop=(ct == 7),
                        )
                    nc.vector.tensor_copy(
                        v_tiles[kb][:, :, 0:64],
                        pr[:].rearrange("p (h d) -> p h d", h=HL),
                    )

                units.append(va)
                units.append(vb)

        def push_outproj_units(s0, cxts):
            for ob in range(2):
                for sq in range(4):

                    def u(ob=ob, sq=sq, s0=s0, cxts=cxts):
                        opp = pp.tile([128, 512], f32, tag="mm", name="opp")
                        for hp in range(4):
                            nc.tensor.matmul(
                                opp[:],
                                cxts[hp][:, sq * 128 : (sq + 1) * 128],
                                wo_t[:, hp, ob * 512 : (ob + 1) * 512],
                                start=(hp == 0),
                                stop=(hp == 3),
                            )
                        ost = os_p.tile([128, 512], f32, tag="os", name="ost")
                        nc.vector.tensor_copy(ost[:], opp[:])
                        nc.sync.dma_start(
                            outp[
                                s0 + sq * 128 : s0 + (sq + 1) * 128,
                                ob * 512 : (ob + 1) * 512,
                            ],
                            ost[:],
                        )

                    units.append(u)

        # -- main schedule --
        sb_slots = [sb for _ in range(reps) for sb in range(NSB)]
        push_proj_units(0, *pd0)
        inject(len(units))  # first projection stands alone
        prev_out = None
        pending_norm = [None]

        def flush_norm():
            if pending_norm[0] is not None:
                f = pending_norm[0]
                pending_norm[0] = None
                f()

        for slot, sb in enumerate(sb_slots):
            s0 = sb * 512
            if slot + 1 < len(sb_slots):
                nsb = sb_slots[slot + 1]
                pd = emit_proj_dmas(nsb)
                push_proj_units(nsb, *pd)
            if prev_out is not None:
                push_outproj_units(*prev_out)
                prev_out = None

            qt_tiles = dict(qt_cur)
            n_kj = 4 * (sb + 1)
            inj_total = len(units)
            steps_total = 4 * (n_kj + DEPTH)
            step = 0
            injected = 0

            def pace():
                nonlocal step, injected
                step += 1
                want = inj_total * step // steps_total
                inject(want - injected)
                injected = want

            cx_tiles = {}
            for hp in range(4):
                pcx = pc.tile([128, 2, 512], f32, tag="cxp", name="cxp")
                exts = {}
                spans = {}

                def emit_v(kj, hp=hp, pcx=pcx, exts=exts, spans=spans, n_kj=n_kj):
                    qo, w = spans[kj]
                    for par in (0, 1):
                        nc.tensor.matmul(
                            pcx[0:65, par, qo : qo + w],
                            v_tiles[kj][:, hp * 2 + par, :],
                            exts[kj][:, par, 0:w],
                            start=(kj == 0),
                            stop=(kj == n_kj - 1),
                        )
                    del exts[kj]

                for kj in range(n_kj):
                    diag = kj >= 4 * sb
                    kjl = kj - 4 * sb
                    w = max(512 - 128 * kjl, 256) if diag else 512
                    qo = 512 - w
                    moff = 384 - (128 * kjl - qo)
                    sbk, col = kj // 4, (kj % 4) * 128
                    spans[kj] = (qo, w)
                    ext = ex_p.tile([128, 2, 512], f32r, tag="ex", name="ex")
                    for par in (0, 1):
                        scp = ps.tile([128, 512], f32, tag="sc", name="sc")
                        bp = 64 * par
                        kt_sl = kt_tiles[hp, sbk][bp : bp + 64, col : col + 128]
                        qt_sl = qt_tiles[hp][bp : bp + 64, qo : qo + w]
                        if diag:
                            tw = 128 if kjl < 3 else 256
                            nc.tensor.matmul(
                                scp[:, qo : qo + w], kt_sl, qt_sl,
                                start=True, stop=False,
                            )
                            nc.tensor.matmul(
                                scp[:, qo : qo + tw],
                                ident_t[:],
                                maskT_t[:, moff : moff + tw],
                                start=False,
                                stop=True,
                            )
                        else:
                            nc.tensor.matmul(
                                scp[:], kt_sl, qt_sl, start=True, stop=True
                            )
                        nc.scalar.activation(
                            ext[:, par, 0:w], scp[:, qo : qo + w],
                            AF.Exp, scale=0.125,
                        )
                    exts[kj] = ext
                    if kj >= DEPTH:
                        emit_v(kj - DEPTH)
                    if kj == 1:
                        flush_norm()
                    pace()
                for kj in range(max(0, n_kj - DEPTH), n_kj):
                    emit_v(kj)
                    pace()

                # softmax normalization: reciprocals now (DVE), the PE
                # broadcast + relay copies + muls deferred into the next
                # head-pair's stream (pcx is freed by the muls)
                rc_t = rc_p.tile([65, 2, 512], f32r, tag="rc", name="rc")
                with nc.allow_low_precision(reason="softmax reciprocal to f32r"):
                    nc.vector.reciprocal(rc_t[64:65, 0, :], pcx[64:65, 0, :])
                    nc.vector.reciprocal(rc_t[64:65, 1, :], pcx[64:65, 1, :])

                def norm_rest(hp=hp, pcx=pcx, rc_t=rc_t, cx_tiles=cx_tiles):
                    cxt = cx_p.tile([128, 512], f32r, tag=f"cx{hp}", name=f"cx{hp}")
                    cx_tiles[hp] = cxt
                    rbp0 = pp.tile([64, 512], f32, tag="mm", name="rbp0")
                    nc.tensor.matmul(
                        rbp0[:], ones_t[64:65, :], rc_t[64:65, 0, :],
                        start=True, stop=True,
                    )
                    rbp1 = pp.tile([64, 512], f32, tag="mm", name="rbp1")
                    nc.tensor.matmul(
                        rbp1[:], ones_t[64:65, :], rc_t[64:65, 1, :],
                        start=True, stop=True,
                    )
                    rbs0 = rl_p.tile([64, 512], f32r, tag="rbs0", name="rbs0")
                    nc.vector.tensor_copy(rbs0[:], rbp0[:])
                    nc.vector.tensor_mul(cxt[0:64, :], pcx[0:64, 0, :], rbs0[:])
                    rbs1 = rl_p.tile([64, 512], f32r, tag="rbs1", name="rbs1")
                    nc.vector.tensor_copy(rbs1[:], rbp1[:])
                    rl_t = rl_p.tile([64, 512], f32r, tag="rl", name="rl")
                    nc.vector.tensor_mul(rl_t[:], pcx[0:64, 1, :], rbs1[:])
                    nc.sync.dma_start(cxt[64:128, :], rl_t[:])

                pending_norm[0] = norm_rest

            inject(len(units))
            prev_out = (s0, cx_tiles)

        flush_norm()
        if prev_out is not None:
            push_outproj_units(*prev_out)
            inject(len(units))


def build(S=S_FULL, reps=1, chain=False):
    import concourse.tile as tile
    from concourse import bacc, mybir

    f32 = mybir.dt.float32
    bf16 = mybir.dt.bfloat16
    nc = bacc.Bacc(None, target_bir_lowering=False, debug=False)
    t = {}
    t["xT"] = nc.dram_tensor("xT", [D, S], f32, kind="ExternalInput")
    t["wqT"] = nc.dram_tensor("wqT", [D, DL], f32, kind="ExternalInput")
    t["wkT"] = nc.dram_tensor("wkT", [D, DL], f32, kind="ExternalInput")
    t["wvT"] = nc.dram_tensor("wvT", [D, DL], f32, kind="ExternalInput")
    t["woT"] = nc.dram_tensor("woT", [DL, D], f32, kind="ExternalInput")
    t["tabsc"] = nc.dram_tensor("tabsc", [128, S], f32, kind="ExternalInput")
    t["tabss"] = nc.dram_tensor("tabss", [128, S], bf16, kind="ExternalInput")
    t["maskT"] = nc.dram_tensor("maskT", [128, 896], bf16, kind="ExternalInput")
    t["ident"] = nc.dram_tensor("ident", [128, 128], bf16, kind="ExternalInput")
    t["ones65"] = nc.dram_tensor("ones65", [65, 64], f32, kind="ExternalInput")
    t["outp"] = nc.dram_tensor("outp", [S, D], f32, kind="ExternalOutput")
    if chain:
        t["chain"] = nc.dram_tensor("chain", [128, 128], f32, kind="ExternalInput")
        t["chain_out"] = nc.dram_tensor("chain_out", [128, 128], f32, kind="ExternalOutput")

    with tile.TileContext(nc) as tc:
        _emit(nc, tc, t, S, reps=reps)
        if chain:
            with tc.tile_pool(name="chp", bufs=1) as chp:
                cht = chp.tile([128, 128], mybir.dt.float32, name="cht")
                nc.sync.dma_start(cht[:], t["chain"][:])
                nc.sync.dma_start(t["chain_out"][:], cht[:])
    nc.compile()
    return nc


def prep_inputs(x, Wq, Wk, Wv, Wo, token_positions, S=S_FULL):
    x = np.asarray(x)
    Wq, Wk, Wv, Wo = (np.asarray(a) for a in (Wq, Wk, Wv, Wo))
    pos = np.asarray(token_positions).astype(np.float64)
    inv = ROPE_THETA ** (-np.arange(0, DK, 2, dtype=np.float64) / DK)  # [32]
    ang = pos[:, None] * inv[None, :]  # [S, 32]
    cos = np.cos(ang).astype(np.float32).T  # [32, S]
    sin = np.sin(ang).astype(np.float32).T
    i_of_p = (np.arange(128) % 64) // 2
    c2 = np.ascontiguousarray(cos[i_of_p, :])  # [128, S]
    sgn = np.where(np.arange(128) % 2 == 0, -1.0, 1.0).astype(np.float32)
    import ml_dtypes

    s2m = np.ascontiguousarray((sin[i_of_p, :] * sgn[:, None]).astype(ml_dtypes.bfloat16))

    maskT = np.where(
        np.arange(896)[None, :] >= np.arange(128)[:, None] + 384, 0.0, NEG
    ).astype(ml_dtypes.bfloat16)
    ident = np.eye(128, dtype=ml_dtypes.bfloat16)
    ones65 = np.ones((65, 64), np.float32)

    nb = x.shape[0]
    maps = []
    for c in range(2 * nb):
        b, half = c // 2, c % 2
        rows = slice(half * DL, (half + 1) * DL)
        maps.append(
            {
                "xT": np.ascontiguousarray(x[b].T),
                "wqT": np.ascontiguousarray(Wq[rows].T),
                "wkT": np.ascontiguousarray(Wk[rows].T),
                "wvT": np.ascontiguousarray(Wv[rows].T),
                "woT": np.ascontiguousarray(Wo[:, rows].T),
                "tabsc": c2,
                "tabss": s2m,
                "maskT": maskT,
                "ident": ident,
                "ones65": ones65,
            }
        )
    return maps


def kernel(x, Wq, Wk, Wv, Wo, token_positions):
    from concourse.bass_utils import run_bass_kernel_spmd

    if "nc" not in _CACHE:
        _CACHE["nc"] = build()
    maps = prep_inputs(x, Wq, Wk, Wv, Wo, token_positions)
    res = run_bass_kernel_spmd(_CACHE["nc"], maps, list(range(8)))
    out = np.empty((B, S_FULL, D), np.float32)
    for b in range(B):
        out[b] = res.results[2 * b]["outp"] + res.results[2 * b + 1]["outp"]
    return out


# revision 21
# speedup vs baseline: 1.0730x; 1.0730x over previous
"""Causal multi-head self-attention with RoPE on 8 Trainium2 NeuronCores.

Sharding: batch (4) x head-half (2) -> 8 self-contained cores. Each core
computes Q/K/V projections for its 8 heads, RoPE, causal flash-style
attention (scores kept transposed [key, query] so probs feed the V matmul
with no on-device transpose), and a partial output projection over its 512
context features. The two partial outputs per batch are summed on host
(the "all-reduce after output projection" of the tensor-parallel split).

v2 layout notes:
- The attention kj-stream for super-block sb is ACT(exp)-bound; projection
  matmuls for sb+1 and the deferred output projection of sb-1 are emitted
  interleaved into it (paced units) so PE stays dense while ACT drains exp.
- Q/K (and the RoPE tables' sin half) are bf16: same PE rate, half the DVE
  rope cost on the two pure-bf16 ops, half the SBUF.
- Softmax normalization per head-pair is emitted right after its last PV
  matmul: one wide reciprocal (both pars), two PE broadcasts, two DVE muls
  reading PSUM directly (no ACT relay copy).
- Scores are masked additively via a PE-written -1e30 triangle and
  exponentiated without max-subtraction (scores are bounded); per-query sums
  come free from a ones-column appended to V.
"""

import sys

sys.path.insert(0, "/opt/trn_rl_repo")

import numpy as np

B, S_FULL, D, H = 4, 2048, 1024, 16
DK = 64  # head dim
HL = 8  # heads per core
DL = HL * DK  # 512 local features
ROPE_THETA = 10000.0
NEG = -1.0e30

_CACHE = {}


def _emit(nc, tc, tensors, S, reps=1):
    import concourse.tile as tile  # noqa: F401
    from concourse import mybir
    from contextlib import ExitStack
    from collections import deque

    f32, f32r = mybir.dt.float32, mybir.dt.float32r
    bf16 = mybir.dt.bfloat16
    AF = mybir.ActivationFunctionType
    SWAP = [i ^ 1 for i in range(32)]
    NSB = S // 512  # query super-blocks
    NKB = S // 128  # key blocks
    DEPTH = 3  # kj-iterations the V-matmul trails the score matmul by

    xT, wqT, wkT, wvT, woT = (
        tensors["xT"], tensors["wqT"], tensors["wkT"], tensors["wvT"], tensors["woT"],
    )
    tabsc, tabss = tensors["tabsc"], tensors["tabss"]
    maskT, ident = tensors["maskT"], tensors["ident"]
    ones65, outp = tensors["ones65"], tensors["outp"]

    with ExitStack() as ctx:
        const = ctx.enter_context(tc.tile_pool(name="const", bufs=1))
        wres = ctx.enter_context(tc.tile_pool(name="wres", bufs=1))
        xt_p = ctx.enter_context(tc.tile_pool(name="xt", bufs=9))
        tb_p = ctx.enter_context(tc.tile_pool(name="tb", bufs=2))
        kt_p = ctx.enter_context(tc.tile_pool(name="kt", bufs=1))
        vt_p = ctx.enter_context(tc.tile_pool(name="vt", bufs=1))
        qt_p = ctx.enter_context(tc.tile_pool(name="qt", bufs=2))
        qs_p = ctx.enter_context(tc.tile_pool(name="qs", bufs=2))
        ex_p = ctx.enter_context(tc.tile_pool(name="ex", bufs=DEPTH + 1))
        cx_p = ctx.enter_context(tc.tile_pool(name="cx", bufs=2))
        rc_p = ctx.enter_context(tc.tile_pool(name="rc", bufs=2))
        rl_p = ctx.enter_context(tc.tile_pool(name="rl", bufs=1))
        os_p = ctx.enter_context(tc.tile_pool(name="os", bufs=3))
        pp = ctx.enter_context(tc.tile_pool(name="pp", bufs=2, space="PSUM"))
        ps = ctx.enter_context(tc.tile_pool(name="ps", bufs=4, space="PSUM"))
        pc = ctx.enter_context(tc.tile_pool(name="pc", bufs=1, space="PSUM"))

        woT_r = woT.rearrange("(t p) o -> p t o", p=128)

        # -- initial loads: wk on sync first (first PE work), x on gpsimd --
        def emit_proj_dmas(sb, all_gpsimd=False):
            s0 = sb * 512
            xts = []
            for ct in range(8):
                xt_t = xt_p.tile([128, 512], f32r, tag="xt", name="xt")
                eng = nc.gpsimd if (all_gpsimd or ct % 2 == 0) else nc.sync
                eng.dma_start(
                    xt_t[:], xT[ct * 128 : (ct + 1) * 128, s0 : s0 + 512].bitcast(f32r)
                )
                xts.append(xt_t)
            tbc_t = tb_p.tile([128, 512], f32, tag="tbc", name="tbc")
            nc.sync.dma_start(tbc_t[:], tabsc[:, s0 : s0 + 512])
            tbs_t = tb_p.tile([128, 512], bf16, tag="tbs", name="tbs")
            nc.sync.dma_start(tbs_t[:], tabss[:, s0 : s0 + 512])
            return xts, tbc_t, tbs_t

        wq_t = wres.tile([128, 8, DL], f32r, tag="wq")
        wk_t = wres.tile([128, 8, DL], f32r, tag="wk")
        wv_t = wres.tile([128, 8, DL], f32r, tag="wv")
        wo_t = wres.tile([128, 4, D], f32r, tag="wo")
        for ct in range(8):
            nc.sync.dma_start(
                wk_t[:, ct, :], wkT[ct * 128 : (ct + 1) * 128, :].bitcast(f32r)
            )
        pd0 = emit_proj_dmas(0, all_gpsimd=True)
        for ct in range(8):
            nc.sync.dma_start(
                wq_t[:, ct, :], wqT[ct * 128 : (ct + 1) * 128, :].bitcast(f32r)
            )
        for ct in range(8):
            nc.gpsimd.dma_start(
                wv_t[:, ct, :], wvT[ct * 128 : (ct + 1) * 128, :].bitcast(f32r)
            )
        maskT_t = const.tile([128, 896], bf16, tag="maskT")
        nc.sync.dma_start(maskT_t[:], maskT[:])
        ident_t = const.tile([128, 128], bf16, tag="ident")
        nc.sync.dma_start(ident_t[:], ident[:])
        ones_t = const.tile([65, 64], f32r, tag="ones")
        nc.sync.dma_start(ones_t[:], ones65[:].bitcast(f32r))
        for it in range(4):
            nc.sync.dma_start(wo_t[:, it, :], woT_r[:, it, :].bitcast(f32r))

        # persistent K / V buffers
        kt_tiles = {}
        for hp in range(4):
            for sbk in range(NSB):
                kt_tiles[hp, sbk] = kt_p.tile(
                    [128, 512], bf16, tag=f"kt{hp}_{sbk}", name=f"kt{hp}_{sbk}"
                )
        v_tiles = {}
        for kb in range(NKB):
            v_tiles[kb] = vt_p.tile([128, HL, 65], f32r, tag=f"v{kb}", name=f"v{kb}")
            nc.vector.memset(v_tiles[kb][:, :, 64:65].bitcast(f32), 1.0)

        # -- deferred-work units, injected into the attention stream --
        units = deque()

        def inject(n):
            for _ in range(n):
                if units:
                    units.popleft()()

        qt_cur = {}
        pr_store = {}

        def push_proj_units(sb, xts, tbc_t, tbs_t):
            def rope(pr, dst):
                qs_t = qs_p.tile([128, 512], f32, tag="qs", name="qs")
                nc.vector.stream_shuffle(qs_t[:], pr[:], SWAP)
                nc.vector.tensor_mul(dst[:], pr[:], tbc_t[:])
                qb_t = qs_p.tile([128, 512], bf16, tag="qb", name="qb")
                nc.vector.tensor_mul(qb_t[:], qs_t[:], tbs_t[:])
                nc.vector.tensor_add(dst[:], dst[:], qb_t[:])

            for w_t, is_k in ((wk_t, True), (wq_t, False)):
                for ft in range(4):
                    key = (sb, is_k, ft)

                    def ua(key=key, w_t=w_t, ft=ft, xts=xts):
                        pr = pp.tile([128, 512], f32, tag="mm", name="pr")
                        pr_store[key] = pr
                        for j, ct in enumerate((0, 2, 4, 6)):
                            nc.tensor.matmul(
                                pr[:],
                                w_t[:, ct, ft * 128 : (ft + 1) * 128],
                                xts[ct][:],
                                start=(j == 0),
                                stop=False,
                            )

                    def ub(key=key, w_t=w_t, ft=ft, xts=xts, is_k=is_k, sb=sb):
                        pr = pr_store.pop(key)
                        for j, ct in enumerate((1, 3, 5, 7)):
                            nc.tensor.matmul(
                                pr[:],
                                w_t[:, ct, ft * 128 : (ft + 1) * 128],
                                xts[ct][:],
                                start=False,
                                stop=(j == 3),
                            )
                        if is_k:
                            dst = kt_tiles[ft, sb]
                        else:
                            dst = qt_p.tile(
                                [128, 512], bf16, tag=f"qt{ft}", name=f"qt{ft}"
                            )
                            qt_cur[ft] = dst
                        rope(pr, dst)

                    units.append(ua)
                    units.append(ub)
            for i in range(4):
                kb = sb * 4 + i
                key = (sb, "v", i)

                def va(key=key, i=i, xts=xts):
                    pr = pp.tile([128, 512], f32, tag="mm", name="vp")
                    pr_store[key] = pr
                    for ct in range(4):
                        nc.tensor.matmul(
                            pr[:],
                            xts[ct][:, i * 128 : (i + 1) * 128],
                            wv_t[:, ct, :],
                            start=(ct == 0),
                            stop=False,
                        )

                def vb(key=key, i=i, xts=xts, kb=kb):
                    pr = pr_store.pop(key)
                    for ct in range(4, 8):
                        nc.tensor.matmul(
                            pr[:],
                            xts[ct][:, i * 128 : (i + 1) * 128],
                            wv_t[:, ct, :],
                            start=False,
                            stop=(ct == 7),
                        )
                    nc.vector.tensor_copy(
                        v_tiles[kb][:, :, 0:64],
                        pr[:].rearrange("p (h d) -> p h d", h=HL),
                    )

                units.append(va)
                units.append(vb)

        def push_outproj_units(s0, cxts):
            for ob in range(2):
                for sq in range(4):

                    def u(ob=ob, sq=sq, s0=s0, cxts=cxts):
                        opp = pp.tile([128, 512], f32, tag="mm", name="opp")
                        for hp in range(4):
                            nc.tensor.matmul(
                                opp[:],
                                cxts[hp][:, sq * 128 : (sq + 1) * 128],
                                wo_t[:, hp, ob * 512 : (ob + 1) * 512],
                                start=(hp == 0),
                                stop=(hp == 3),
                            )
                        ost = os_p.tile([128, 512], f32, tag="os", name="ost")
                        nc.vector.tensor_copy(ost[:], opp[:])
                        nc.sync.dma_start(
                            outp[
                                s0 + sq * 128 : s0 + (sq + 1) * 128,
                                ob * 512 : (ob + 1) * 512,
                            ],
                            ost[:],
                        )

                    units.append(u)

        # -- main schedule --
        sb_slots = [sb for _ in range(reps) for sb in range(NSB)]
        push_proj_units(0, *pd0)
        inject(len(units))  # first projection stands alone
        prev_out = None
        pending_norm = [None]

        def flush_norm():
            if pending_norm[0] is not None:
                f = pending_norm[0]
                pending_norm[0] = None
                f()

        for slot, sb in enumerate(sb_slots):
            s0 = sb * 512
            if slot + 1 < len(sb_slots):
                nsb = sb_slots[slot + 1]
                pd = emit_proj_dmas(nsb)
                push_proj_units(nsb, *pd)
            if prev_out is not None:
                push_outproj_units(*prev_out)
                prev_out = None

            qt_tiles = dict(qt_cur)
            n_kj = 4 * (sb + 1)
            inj_total = len(units)
            steps_total = 4 * (n_kj + DEPTH)
            step = 0
            injected = 0

            def pace():
                nonlocal step, injected
                step += 1
                want = inj_total * step // steps_total
                inject(want - injected)
                injected = want

            cx_tiles = {}
            for hp in range(4):
                pcx = pc.tile([128, 2, 512], f32, tag="cxp", name="cxp")
                exts = {}
                spans = {}

                def emit_v(kj, hp=hp, pcx=pcx, exts=exts, spans=spans, n_kj=n_kj):
                    qo, w = spans[kj]
                    for par in (0, 1):
                        nc.tensor.matmul(
                            pcx[0:65, par, qo : qo + w],
                            v_tiles[kj][:, hp * 2 + par, :],
                            exts[kj][:, par, 0:w],
                            start=(kj == 0),
                            stop=(kj == n_kj - 1),
                        )
                    del exts[kj]

                for kj in range(n_kj):
                    diag = kj >= 4 * sb
                    kjl = kj - 4 * sb
                    w = max(512 - 128 * kjl, 256) if diag else 512
                    qo = 512 - w
                    moff = 384 - (128 * kjl - qo)
                    sbk, col = kj // 4, (kj % 4) * 128
                    spans[kj] = (qo, w)
                    ext = ex_p.tile([128, 2, 512], f32r, tag="ex", name="ex")
                    scps = []
                    for par in (0, 1):
                        scp = ps.tile([128, 512], f32, tag="sc", name="sc")
                        scps.append(scp)
                        bp = 64 * par
                        kt_sl = kt_tiles[hp, sbk][bp : bp + 64, col : col + 128]
                        qt_sl = qt_tiles[hp][bp : bp + 64, qo : qo + w]
                        if diag:
                            tw = 128 if kjl < 3 else 256
                            nc.tensor.matmul(
                                scp[:, qo : qo + w], kt_sl, qt_sl,
                                start=True, stop=False,
                            )
                            nc.tensor.matmul(
                                scp[:, qo : qo + tw],
                                ident_t[:],
                                maskT_t[:, moff : moff + tw],
                                start=False,
                                stop=True,
                            )
                        else:
                            nc.tensor.matmul(
                                scp[:], kt_sl, qt_sl, start=True, stop=True
                            )
                    for par in (0, 1):
                        nc.scalar.activation(
                            ext[:, par, 0:w], scps[par][:, qo : qo + w],
                            AF.Exp, scale=0.125,
                        )
                    exts[kj] = ext
                    if kj >= DEPTH:
                        emit_v(kj - DEPTH)
                    if kj == 1:
                        flush_norm()
                    pace()
                for kj in range(max(0, n_kj - DEPTH), n_kj):
                    emit_v(kj)
                    pace()

                # softmax normalization: reciprocals now (DVE), the PE
                # broadcast + relay copies + muls deferred into the next
                # head-pair's stream (pcx is freed by the muls)
                rc_t = rc_p.tile([65, 2, 512], f32r, tag="rc", name="rc")
                with nc.allow_low_precision(reason="softmax reciprocal to f32r"):
                    nc.vector.reciprocal(rc_t[64:65, 0, :], pcx[64:65, 0, :])
                    nc.vector.reciprocal(rc_t[64:65, 1, :], pcx[64:65, 1, :])

                def norm_rest(hp=hp, pcx=pcx, rc_t=rc_t, cx_tiles=cx_tiles):
                    cxt = cx_p.tile([128, 512], f32r, tag=f"cx{hp}", name=f"cx{hp}")
                    cx_tiles[hp] = cxt
                    rbp0 = pp.tile([64, 512], f32, tag="mm", name="rbp0")
                    nc.tensor.matmul(
                        rbp0[:], ones_t[64:65, :], rc_t[64:65, 0, :],
                        start=True, stop=True,
                    )
                    rbp1 = pp.tile([64, 512], f32, tag="mm", name="rbp1")
                    nc.tensor.matmul(
                        rbp1[:], ones_t[64:65, :], rc_t[64:65, 1, :],
                        start=True, stop=True,
                    )
                    rbs0 = rl_p.tile([64, 512], f32r, tag="rbs0", name="rbs0")
                    nc.vector.tensor_copy(rbs0[:], rbp0[:])
                    nc.vector.tensor_mul(cxt[0:64, :], pcx[0:64, 0, :], rbs0[:])
                    rbs1 = rl_p.tile([64, 512], f32r, tag="rbs1", name="rbs1")
                    nc.vector.tensor_copy(rbs1[:], rbp1[:])
                    rl_t = rl_p.tile([64, 512], f32r, tag="rl", name="rl")
                    nc.vector.tensor_mul(rl_t[:], pcx[0:64, 1, :], rbs1[:])
                    nc.sync.dma_start(cxt[64:128, :], rl_t[:])

                pending_norm[0] = norm_rest

            inject(len(units))
            prev_out = (s0, cx_tiles)

        flush_norm()
        if prev_out is not None:
            push_outproj_units(*prev_out)
            inject(len(units))


def build(S=S_FULL, reps=1, chain=False):
    import concourse.tile as tile
    from concourse import bacc, mybir

    f32 = mybir.dt.float32
    bf16 = mybir.dt.bfloat16
    nc = bacc.Bacc(None, target_bir_lowering=False, debug=False)
    t = {}
    t["xT"] = nc.dram_tensor("xT", [D, S], f32, kind="ExternalInput")
    t["wqT"] = nc.dram_tensor("wqT", [D, DL], f32, kind="ExternalInput")
    t["wkT"] = nc.dram_tensor("wkT", [D, DL], f32, kind="ExternalInput")
    t["wvT"] = nc.dram_tensor("wvT", [D, DL], f32, kind="ExternalInput")
    t["woT"] = nc.dram_tensor("woT", [DL, D], f32, kind="ExternalInput")
    t["tabsc"] = nc.dram_tensor("tabsc", [128, S], f32, kind="ExternalInput")
    t["tabss"] = nc.dram_tensor("tabss", [128, S], bf16, kind="ExternalInput")
    t["maskT"] = nc.dram_tensor("maskT", [128, 896], bf16, kind="ExternalInput")
    t["ident"] = nc.dram_tensor("ident", [128, 128], bf16, kind="ExternalInput")
    t["ones65"] = nc.dram_tensor("ones65", [65, 64], f32, kind="ExternalInput")
    t["outp"] = nc.dram_tensor("outp", [S, D], f32, kind="ExternalOutput")
    if chain:
        t["chain"] = nc.dram_tensor("chain", [128, 128], f32, kind="ExternalInput")
        t["chain_out"] = nc.dram_tensor("chain_out", [128, 128], f32, kind="ExternalOutput")

    with tile.TileContext(nc) as tc:
        _emit(nc, tc, t, S, reps=reps)
        if chain:
            with tc.tile_pool(name="chp", bufs=1) as chp:
                cht = chp.tile([128, 128], mybir.dt.float32, name="cht")
                nc.sync.dma_start(cht[:], t["chain"][:])
                nc.sync.dma_start(t["chain_out"][:], cht[:])
    nc.compile()
    return nc


def prep_inputs(x, Wq, Wk, Wv, Wo, token_positions, S=S_FULL):
    x = np.asarray(x)
    Wq, Wk, Wv, Wo = (np.asarray(a) for a in (Wq, Wk, Wv, Wo))
    pos = np.asarray(token_positions).astype(np.float64)
    inv = ROPE_THETA ** (-np.arange(0, DK, 2, dtype=np.float64) / DK)  # [32]
    ang = pos[:, None] * inv[None, :]  # [S, 32]
    cos = np.cos(ang).astype(np.float32).T  # [32, S]
    sin = np.sin(ang).astype(np.float32).T
    i_of_p = (np.arange(128) % 64) // 2
    c2 = np.ascontiguousarray(cos[i_of_p, :])  # [128, S]
    sgn = np.where(np.arange(128) % 2 == 0, -1.0, 1.0).astype(np.float32)
    import ml_dtypes

    s2m = np.ascontiguousarray((sin[i_of_p, :] * sgn[:, None]).astype(ml_dtypes.bfloat16))

    maskT = np.where(
        np.arange(896)[None, :] >= np.arange(128)[:, None] + 384, 0.0, NEG
    ).astype(ml_dtypes.bfloat16)
    ident = np.eye(128, dtype=ml_dtypes.bfloat16)
    ones65 = np.ones((65, 64), np.float32)

    nb = x.shape[0]
    maps = []
    for c in range(2 * nb):
        b, half = c // 2, c % 2
        rows = slice(half * DL, (half + 1) * DL)
        maps.append(
            {
                "xT": np.ascontiguousarray(x[b].T),
                "wqT": np.ascontiguousarray(Wq[rows].T),
                "wkT": np.ascontiguousarray(Wk[rows].T),
                "wvT": np.ascontiguousarray(Wv[rows].T),
                "woT": np.ascontiguousarray(Wo[:, rows].T),
                "tabsc": c2,
                "tabss": s2m,
                "maskT": maskT,
                "ident": ident,
                "ones65": ones65,
            }
        )
    return maps


def kernel(x, Wq, Wk, Wv, Wo, token_positions):
    from concourse.bass_utils import run_bass_kernel_spmd

    if "nc" not in _CACHE:
        _CACHE["nc"] = build()
    maps = prep_inputs(x, Wq, Wk, Wv, Wo, token_positions)
    res = run_bass_kernel_spmd(_CACHE["nc"], maps, list(range(8)))
    out = np.empty((B, S_FULL, D), np.float32)
    for b in range(B):
        out[b] = res.results[2 * b]["outp"] + res.results[2 * b + 1]["outp"]
    return out
